# revision 19
# baseline (speedup 1.0000x reference)
"""Trainium2 Bass kernel for nn_DualStateLinearAttention (v2).

Reference math (B=2, S=2048, HID=2048, H=16, D=128):
    q = x @ Wq.T, k = x @ Wk.T, v = x @ Wv.T            (split into 16 heads)
    gk_j = clamp(log_sigmoid(x @ Wgj.T + bgj) / 16, min=-50)   j in {1,2}
    o_j  = GLA scan over S with per-key-dim decay exp(gk_j)
    out  = (softmax(alpha)[0] * o1 + softmax(alpha)[1] * o2) @ Wo.T

Strategy (8 NeuronCores, tensor-parallel over heads):
  - 2 heads per core; q/k/v/gate projections column-parallel, o_proj
    row-parallel; each core emits a partial [B*S, HID] fp16 output which
    the host sums (the all-reduce of row-parallel o_proj).
  - GLA evaluated in chunked form (chunk C=128), all matmul operands fp16
    with fp32 PSUM accumulation.  No mid-chunk decay shift: the
    within-chunk gate cumsum G spans <= ~e^9 for randn inputs, so the
    decayed operands q*e^G (<=|q|) and k*e^-G (<= |k|*e^9 ~ 4e3) sit
    safely inside fp16 range.
  - gate = -softplus(-z)/16 via a single ACT op (AF.Softplus); identity
    gate mode computes all gates up front from x columns in 4 wide
    activations so the ACT engine loads each function table once.
  - PSUM banks: proj x2, o_proj x2, {G-cumsum + K2-transpose} x1,
    {AT + o_t + kv} x3.  The 3-deep shared chain tag means o_t of the
    next chunk never waits on the attn copy of the current one, and the
    scan prologue is never gated on the serial S-chain.
  - DMA queues: weights + out on GpSimd (25ns dispatch), x tiles on
    Sync, gate x-columns on Vector.  The ACT queue carries no DMA.
"""

import os
import sys

import numpy as np

for _p in ("/opt/trn_rl_repo",):
    if os.path.isdir(_p) and _p not in sys.path:
        sys.path.insert(0, _p)

import concourse.bass as bass
import concourse.mybir as mybir
import concourse.tile as tile
from concourse import bacc
from concourse.bass_utils import run_bass_kernel_spmd

F32 = mybir.dt.float32
F16 = mybir.dt.float16
AF = mybir.ActivationFunctionType
OP = mybir.AluOpType

B, S, HID = 2, 2048, 2048
H, DH = 16, 128
NCORES = 8
HPC = H // NCORES          # heads per core
DC = HPC * DH              # per-core head dims (256)
TOK = B * S
SLAB = 512
CHUNK = 128
GATE_NORM = 16.0
CLAMP_MIN = -50.0


def build_nc(tok=TOK, gate_mode="identity"):
    """Build the per-core SPMD Bass program.

    gate_mode: "identity" -> gate preactivation is x columns (no projection)
               "general"  -> gate = x @ Wg.T + bg computed on device
    """
    assert tok % SLAB == 0 and (tok // B) % SLAB == 0
    nslabs = tok // SLAB
    spb = (tok // B) // SLAB   # slabs per batch
    tpb = tok // B             # tokens per batch
    n_ct = HID // 128          # contraction tiles
    n_tt = SLAB // CHUNK       # token chunks per slab
    n_eo = HID // 512          # output column tiles
    ident_gate = gate_mode == "identity"

    nc = bacc.Bacc(None, target_bir_lowering=False, debug=False)

    xT = nc.dram_tensor("xT", [HID, tok], F16, kind="ExternalInput")
    wqT = nc.dram_tensor("wqT", [HID, DC], F16, kind="ExternalInput")
    wkT = nc.dram_tensor("wkT", [HID, DC], F16, kind="ExternalInput")
    wvT = nc.dram_tensor("wvT", [HID, DC], F16, kind="ExternalInput")
    woT = nc.dram_tensor("woT", [DC, HID], F16, kind="ExternalInput")
    u1 = nc.dram_tensor("u1", [CHUNK, CHUNK], F16, kind="ExternalInput")
    ident = nc.dram_tensor("ident", [CHUNK, CHUNK], F16, kind="ExternalInput")
    if ident_gate:
        xg = nc.dram_tensor("xg", [tok, DC], F16, kind="ExternalInput")
    else:
        wgT = nc.dram_tensor("wgT", [HID, DC], F16, kind="ExternalInput")
        bg = nc.dram_tensor("bg", [1, DC], F16, kind="ExternalInput")
    out = nc.dram_tensor("out", [tok, HID], F16, kind="ExternalOutput")

    # interleave batches: consecutive same-batch slabs have serially
    # dependent scans; alternating keeps two recurrent sweeps in flight.
    slab_order = [bb * spb + si for si in range(spb) for bb in range(B)]

    from contextlib import ExitStack

    with tile.TileContext(nc) as tc, ExitStack() as _st:
        def _pool(name, bufs, space=None):
            kw = {"space": space} if space is not None else {}
            return _st.enter_context(tc.tile_pool(name=name, bufs=bufs, **kw))

        PSUM = bass.MemorySpace.PSUM
        consts = _pool("consts", 1)
        xtp = _pool("xtp", 3)
        gxp = _pool("gxp", 2)
        ggp = _pool("ggp", 2 if ident_gate else 3)
        qkp = _pool("qkp", 3)
        vp = _pool("vp", 8)
        egp = _pool("egp", 4)
        engp = _pool("engp", 3)
        qtp = _pool("qtp", 4)
        ktp = _pool("ktp", 3)
        k2dp = _pool("k2dp", 3)
        k2tp = _pool("k2tp", 16)
        atmp = _pool("atmp", 16)
        attnp = _pool("attnp", 2)
        ostp = _pool("ostp", 2)
        statep = _pool("statep", 2)
        pspp = _pool("ps_pp", 2, PSUM)
        pso = _pool("ps_o", 2, PSUM)
        psgt = _pool("ps_gt", 1, PSUM)
        pssc = _pool("ps_sc", 3, PSUM)
        if True:
            # ---- constants (GpSimd queue: ~25ns dispatch each) ----
            wq_sb = consts.tile([128, n_ct, DC], F16)
            wk_sb = consts.tile([128, n_ct, DC], F16)
            wv_sb = consts.tile([128, n_ct, DC], F16)
            wo_sb = consts.tile([128, HPC, HID], F16)
            def load_w(dst, src, split):
                step = n_ct // split
                for c0 in range(0, n_ct, step):
                    nc.gpsimd.dma_start(
                        dst[:, c0:c0 + step, :],
                        src[c0 * 128:(c0 + step) * 128, :].rearrange(
                            "(ct p) d -> p ct d", p=128
                        ),
                    )

            load_w(wv_sb, wvT, 4)
            load_w(wq_sb, wqT, 2)
            load_w(wk_sb, wkT, 2)
            u1_sb = consts.tile([CHUNK, CHUNK], F16)
            nc.gpsimd.dma_start(u1_sb, u1[:, :])
            id_sb = consts.tile([CHUNK, CHUNK], F16)
            nc.gpsimd.dma_start(id_sb, ident[:, :])
            nc.gpsimd.dma_start(
                wo_sb, woT[:, :].rearrange("(hp p) e -> p hp e", p=128)
            )
            if not ident_gate:
                wg_sb = consts.tile([128, n_ct, DC], F16)
                nc.gpsimd.dma_start(
                    wg_sb, wgT[:, :].rearrange("(ct p) d -> p ct d", p=128)
                )
                bg_sb = consts.tile([1, DC], F16)
                nc.gpsimd.dma_start(bg_sb, bg[:, :])
                ones1 = consts.tile([1, CHUNK], F16)
                nc.vector.memset(ones1, 1.0)

            # ---- per-(batch, head) recurrent state [dk, dv], fp16 ----
            s_tiles = {}
            for bh in range(B * HPC):
                t = statep.tile([DH, DH], F16, tag=f"S{bh}")
                nc.vector.memset(t.bitcast(F32), 0.0)
                s_tiles[bh] = t

            # ---- x slab tiles (Sync queue) ----
            xts_map = {}

            def load_x(slab, split=1):
                # one slab of x as a single [128, n_ct, SLAB] tile; `split`
                # chops the DMA so the first ct-tiles land early (slab 0)
                t0 = slab * SLAB
                t = xtp.tile([128, n_ct, SLAB], F16, tag="xt", name=f"x{slab}")
                step = n_ct // split
                for c0 in range(0, n_ct, step):
                    nc.sync.dma_start(
                        t[:, c0:c0 + step, :],
                        xT[c0 * 128:(c0 + step) * 128, t0:t0 + SLAB].rearrange(
                            "(ct p) t -> p ct t", p=128
                        ),
                    )
                xts_map[slab] = [t[:, ct, :] for ct in range(n_ct)]

            load_x(slab_order[0], split=4)
            load_x(slab_order[1], split=4)

            # ---- identity-gate precompute: per-batch x-column tiles,
            #      4 wide softplus ACTs (one table load), per-slab TS ----
            g_tiles = {}
            sp_tiles = {}
            if ident_gate:
                for b in range(B):
                    xg_sb = gxp.tile([128, spb * n_tt, DC], F16, tag="xg")
                    nc.scalar.dma_start(
                        xg_sb,
                        xg[b * tpb:(b + 1) * tpb, :].rearrange(
                            "(c p) d -> p c d", p=128
                        ),
                    )
                    ex_sb = gxp.tile([128, spb * n_tt, DC], F16, tag="ex")
                    ln_sb = gxp.tile([128, spb * n_tt, DC], F16, tag="lns")
                    sp_tiles[b] = (xg_sb, ex_sb, ln_sb)
                    g_tiles[b] = ggp.tile(
                        [128, spb * n_tt, DC], F16, tag="g", name=f"g{b}"
                    )
                half = (spb * n_tt) // 2

                def emit_exp(hi, b):
                    xg_sb, ex_sb, _ = sp_tiles[b]
                    blk = slice(hi * half, (hi + 1) * half)
                    nc.scalar.activation(
                        ex_sb[:, blk, :], xg_sb[:, blk, :], AF.Exp, scale=-1.0
                    )

                def emit_ln(hi, b):
                    # softplus(-z) = ln(1 + exp(-z)); scaling/clamp folded
                    # into the per-slab tensor_scalar below
                    _, ex_sb, ln_sb = sp_tiles[b]
                    blk = slice(hi * half, (hi + 1) * half)
                    nc.scalar.activation(
                        ln_sb[:, blk, :], ex_sb[:, blk, :], AF.Ln, bias=1.0
                    )

                # all gate ACT up front: slab 0's PE-critical copies are on
                # the vector queue (v), so ~15us of gate work on ACT hides
                # behind the v/q/k projections; 3 table loads total
                emit_exp(0, 0)
                emit_exp(0, 1)
                emit_exp(1, 0)
                emit_exp(1, 1)
                emit_ln(0, 0)
                emit_ln(0, 1)
                emit_ln(1, 0)
                emit_ln(1, 1)

            # ---- main slab loop ----
            for idx, slab in enumerate(slab_order):
                b = slab // spb
                lo = slab % spb
                t0 = slab * SLAB
                if idx + 2 < nslabs:
                    load_x(slab_order[idx + 2], split=4 if idx == 0 else 1)
                xts = xts_map.pop(slab)

                if ident_gate:
                    g_b = g_tiles[b]
                    blk = slice(lo * n_tt, (lo + 1) * n_tt)
                    nc.vector.tensor_scalar(
                        g_b[:, blk, :], sp_tiles[b][2][:, blk, :],
                        -1.0 / GATE_NORM, CLAMP_MIN, op0=OP.mult, op1=OP.max,
                    )

                    def gsl(ci, h, _g=g_b, _lo=lo):
                        return _g[:, _lo * n_tt + ci, h * DH:(h + 1) * DH]

                # T-major v projection (out [t, d]); copies on Vector
                v_tiles = []
                for tt in range(n_tt):
                    ps = pspp.tile([128, DC], F32, tag="pp")
                    for ct in range(n_ct):
                        nc.tensor.matmul(
                            ps,
                            xts[ct][:, tt * CHUNK:(tt + 1) * CHUNK],
                            wv_sb[:, ct, :],
                            start=(ct == 0),
                            stop=(ct == n_ct - 1),
                        )
                    v_sb = vp.tile([128, DC], F16, tag="v")
                    nc.vector.tensor_copy(v_sb, ps)
                    v_tiles.append(v_sb)

                # D-major q/k projections (out [d, t])
                qsb, ksb = {}, {}
                for h in range(HPC):
                    for name, wsb, store in (("q", wq_sb, qsb), ("k", wk_sb, ksb)):
                        ps = pspp.tile([128, SLAB], F32, tag="pp")
                        for ct in range(n_ct):
                            nc.tensor.matmul(
                                ps,
                                wsb[:, ct, h * DH:(h + 1) * DH],
                                xts[ct],
                                start=(ct == 0),
                                stop=(ct == n_ct - 1),
                            )
                        sbt = qkp.tile([128, SLAB], F32, tag=name)
                        nc.scalar.copy(sbt, ps)
                        store[h] = sbt

                # general-gate: z = x @ Wg.T + bg, T-major, then softplus
                if not ident_gate:
                    g_t = ggp.tile([128, n_tt, DC], F16, tag="g")

                    def gsl(ci, h, _g=g_t):
                        return _g[:, ci, h * DH:(h + 1) * DH]

                    for tt in range(n_tt):
                        zps = pspp.tile([128, DC], F32, tag="pp")
                        for ct in range(n_ct):
                            nc.tensor.matmul(
                                zps,
                                xts[ct][:, tt * CHUNK:(tt + 1) * CHUNK],
                                wg_sb[:, ct, :],
                                start=(ct == 0),
                                stop=False,
                            )
                        nc.tensor.matmul(zps, ones1, bg_sb, start=False, stop=True)
                        spe = gxp.tile([128, DC], F16, tag="spe")
                        nc.scalar.activation(spe, zps, AF.Exp, scale=-1.0)
                        spt = gxp.tile([128, DC], F16, tag="spt")
                        nc.scalar.activation(spt, spe, AF.Ln, bias=1.0)
                        nc.vector.tensor_scalar(
                            g_t[:, tt, :], spt,
                            -1.0 / GATE_NORM, CLAMP_MIN, op0=OP.mult, op1=OP.max,
                        )

                # G cumsum matmuls + exps: expG/expNG assembled per head
                # into [128, SLAB] tiles so qt/kt are single wide muls
                egs, engs = {}, {}
                for h in range(HPC):
                    egA = egp.tile([128, SLAB], F32, tag="eg")
                    engA = engp.tile([128, SLAB], F32, tag="eng")
                    for ci in range(n_tt):
                        gps = psgt.tile([DH, CHUNK], F32, tag="gt")
                        nc.tensor.matmul(gps, gsl(ci, h), u1_sb, start=True, stop=True)
                        cs = slice(ci * CHUNK, (ci + 1) * CHUNK)
                        nc.scalar.activation(egA[:, cs], gps, AF.Exp)
                        nc.scalar.activation(engA[:, cs], gps, AF.Exp, scale=-1.0)
                    egs[h], engs[h] = egA, engA

                # decayed q/k + per-chunk K2 transpose and AT
                qts, k2ts, atms = {}, {}, {}
                for h in range(HPC):
                    qtA = qtp.tile([128, SLAB], F16, tag="qt")
                    nc.vector.tensor_mul(qtA, qsb[h], egs[h])
                    ktA = ktp.tile([128, SLAB], F16, tag="kt")
                    nc.vector.tensor_mul(ktA, ksb[h], engs[h])
                    qts[h] = qtA
                    for ci in range(n_tt):
                        cs = slice(ci * CHUNK, (ci + 1) * CHUNK)
                        last = slice(ci * CHUNK + CHUNK - 1, ci * CHUNK + CHUNK)
                        k2d = k2dp.tile([DH, CHUNK], F16, tag="k2d")
                        nc.vector.tensor_scalar_mul(k2d, ktA[:, cs], egs[h][:, last])
                        tps = psgt.tile([CHUNK, DH], F16, tag="gt")
                        nc.tensor.transpose(tps, k2d, id_sb)
                        k2t = k2tp.tile([CHUNK, DH], F16, tag="k2t")
                        nc.vector.tensor_copy(k2t, tps)
                        k2ts[(h, ci)] = k2t
                        aps = pssc.tile([CHUNK, CHUNK], F32, tag="sc")
                        nc.tensor.matmul(aps, ktA[:, cs], qtA[:, cs], start=True, stop=True)
                        atm = atmp.tile([CHUNK, CHUNK], F16, tag="atm")
                        nc.vector.tensor_mul(atm, aps, u1_sb)
                        atms[(h, ci)] = atm

                # recurrent sweep (chunk-serial per head) + o_proj per chunk
                attn_tiles = {}
                for h in range(HPC):
                    attn_tiles[h] = attnp.tile(
                        [DH, SLAB], F16, tag=f"at{h}", name=f"attn{h}"
                    )
                for ci in range(n_tt):
                    cs = slice(ci * CHUNK, (ci + 1) * CHUNK)
                    last = slice(ci * CHUNK + CHUNK - 1, ci * CHUNK + CHUNK)
                    for h in range(HPC):
                        bh = b * HPC + h
                        s_old = s_tiles[bh]
                        v_T = v_tiles[ci][:, h * DH:(h + 1) * DH]
                        ot = pssc.tile([DH, CHUNK], F32, tag="sc")
                        nc.tensor.matmul(ot, s_old, qts[h][:, cs], start=True, stop=False)
                        nc.tensor.matmul(ot, v_T, atms[(h, ci)], start=False, stop=True)
                        nc.scalar.copy(attn_tiles[h][:, cs], ot)
                        kvp = pssc.tile([DH, DH], F32, tag="sc")
                        nc.tensor.matmul(kvp, k2ts[(h, ci)], v_T, start=True, stop=True)
                        s_new = statep.tile([DH, DH], F16, tag=f"S{bh}")
                        nc.vector.scalar_tensor_tensor(
                            s_new, s_old, egs[h][:, last], kvp,
                            op0=OP.mult, op1=OP.add,
                        )
                        s_tiles[bh] = s_new

                    # row-parallel o_proj for this chunk's tokens
                    ostage = ostp.tile([CHUNK, HID], F16, tag="os")
                    for eo in range(n_eo):
                        ops = pso.tile([CHUNK, 512], F32, tag="o")
                        for h in range(HPC):
                            nc.tensor.matmul(
                                ops,
                                attn_tiles[h][:, cs],
                                wo_sb[:, h, eo * 512:(eo + 1) * 512],
                                start=(h == 0),
                                stop=(h == HPC - 1),
                            )
                        dst = ostage[:, eo * 512:(eo + 1) * 512]
                        if eo % 2:
                            nc.scalar.copy(dst, ops)
                        else:
                            nc.vector.tensor_copy(dst, ops)
                    nc.gpsimd.dma_start(
                        out[t0 + ci * CHUNK:t0 + (ci + 1) * CHUNK, :], ostage
                    )
    nc.compile()
    return nc


_NC_CACHE = {}
LAST_RESULTS = []


def _get_nc(tok, gate_mode):
    key = (tok, gate_mode)
    if key not in _NC_CACHE:
        _NC_CACHE[key] = build_nc(tok, gate_mode)
    return _NC_CACHE[key]


def _make_in_maps(xT, x, Wq, Wk, Wv, Wo, gate_mode, Wg=None, bgv=None, tok=TOK):
    scale = DH ** -0.5
    u1m = np.triu(np.ones((CHUNK, CHUNK), np.float16))
    idm = np.eye(CHUNK, dtype=np.float16)
    xTp = np.ascontiguousarray(xT.astype(np.float16))
    in_maps = []
    for c in range(NCORES):
        rs = slice(c * DC, (c + 1) * DC)
        m = dict(
            xT=xTp,
            wqT=np.ascontiguousarray((Wq[rs] * scale).T.astype(np.float16)),
            wkT=np.ascontiguousarray(Wk[rs].T.astype(np.float16)),
            wvT=np.ascontiguousarray(Wv[rs].T.astype(np.float16)),
            woT=np.ascontiguousarray(Wo[:, rs].T.astype(np.float16)),
            u1=u1m,
            ident=idm,
        )
        if gate_mode == "identity":
            m["xg"] = np.ascontiguousarray(x[:, rs].astype(np.float16))
        else:
            m["wgT"] = np.ascontiguousarray(Wg[rs].T.astype(np.float16))
            m["bg"] = np.ascontiguousarray(bgv[rs].astype(np.float16)).reshape(1, DC)
        in_maps.append(m)
    return in_maps


def _run(nc, in_maps):
    trace = bool(int(os.environ.get("GLA_TRACE", "0")))
    res = run_bass_kernel_spmd(
        nc, in_maps, list(range(NCORES)), trace=trace,
    )
    LAST_RESULTS.append(res)
    total = res.results[0]["out"].astype(np.float32)
    for i in range(1, NCORES):
        total += res.results[i]["out"].astype(np.float32)
    return total


def kernel(hidden_states, Wq, Wk, Wv, Wo, Wg1, bg1, Wg2, bg2, alpha_list):
    LAST_RESULTS.clear()
    x = np.ascontiguousarray(np.asarray(hidden_states, np.float32).reshape(TOK, HID))
    xT = np.ascontiguousarray(x.T)
    Wq = np.asarray(Wq, np.float32)
    Wk = np.asarray(Wk, np.float32)
    Wv = np.asarray(Wv, np.float32)
    Wo = np.asarray(Wo, np.float32)
    Wg1 = np.asarray(Wg1, np.float32)
    Wg2 = np.asarray(Wg2, np.float32)
    bg1 = np.asarray(bg1, np.float32)
    bg2 = np.asarray(bg2, np.float32)
    al = np.asarray(alpha_list, np.float64)
    a = np.exp(al - al.max())
    a = (a / a.sum()).astype(np.float32)

    gates_equal = np.array_equal(Wg1, Wg2) and np.array_equal(bg1, bg2)
    ident_gate = (
        gates_equal
        and not bg1.any()
        and np.array_equal(Wg1, np.eye(HID, dtype=np.float32))
    )

    if ident_gate:
        nc = _get_nc(TOK, "identity")
        out = _run(nc, _make_in_maps(xT, x, Wq, Wk, Wv, Wo, "identity"))
    elif gates_equal:
        nc = _get_nc(TOK, "general")
        out = _run(nc, _make_in_maps(xT, x, Wq, Wk, Wv, Wo, "general", Wg1, bg1))
    else:
        nc = _get_nc(TOK, "general")
        o1 = _run(nc, _make_in_maps(xT, x, Wq, Wk, Wv, Wo, "general", Wg1, bg1))
        o2 = _run(nc, _make_in_maps(xT, x, Wq, Wk, Wv, Wo, "general", Wg2, bg2))
        out = a[0] * o1 + a[1] * o2

    return out.reshape(B, S, HID)


# revision 22
# speedup vs baseline: 1.1896x; 1.1896x over previous
"""Trainium2 Bass kernel for nn_DualStateLinearAttention (v2).

Reference math (B=2, S=2048, HID=2048, H=16, D=128):
    q = x @ Wq.T, k = x @ Wk.T, v = x @ Wv.T            (split into 16 heads)
    gk_j = clamp(log_sigmoid(x @ Wgj.T + bgj) / 16, min=-50)   j in {1,2}
    o_j  = GLA scan over S with per-key-dim decay exp(gk_j)
    out  = (softmax(alpha)[0] * o1 + softmax(alpha)[1] * o2) @ Wo.T

Strategy (8 NeuronCores, tensor-parallel over heads):
  - 2 heads per core; q/k/v/gate projections column-parallel, o_proj
    row-parallel; each core emits a partial [B*S, HID] fp16 output which
    the host sums (the all-reduce of row-parallel o_proj).
  - GLA evaluated in chunked form (chunk C=128), all matmul operands fp16
    with fp32 PSUM accumulation.  No mid-chunk decay shift: the
    within-chunk gate cumsum G spans <= ~e^9 for randn inputs, so the
    decayed operands q*e^G (<=|q|) and k*e^-G (<= |k|*e^9 ~ 4e3) sit
    safely inside fp16 range.
  - gate = -softplus(-z)/16 via a single ACT op (AF.Softplus); identity
    gate mode computes all gates up front from x columns in 4 wide
    activations so the ACT engine loads each function table once.
  - PSUM banks: proj x2, o_proj x2, {G-cumsum + K2-transpose} x1,
    {AT + o_t + kv} x3.  The 3-deep shared chain tag means o_t of the
    next chunk never waits on the attn copy of the current one, and the
    scan prologue is never gated on the serial S-chain.
  - DMA queues: weights + out on GpSimd (25ns dispatch), x tiles on
    Sync, gate x-columns on Vector.  The ACT queue carries no DMA.
"""

import os
import sys

import numpy as np

for _p in ("/opt/trn_rl_repo",):
    if os.path.isdir(_p) and _p not in sys.path:
        sys.path.insert(0, _p)

import concourse.bass as bass
import concourse.mybir as mybir
import concourse.tile as tile
from concourse import bacc
from concourse.bass_utils import run_bass_kernel_spmd

F32 = mybir.dt.float32
F16 = mybir.dt.float16
AF = mybir.ActivationFunctionType
OP = mybir.AluOpType

B, S, HID = 2, 2048, 2048
H, DH = 16, 128
NCORES = 8
HPC = H // NCORES          # heads per core
DC = HPC * DH              # per-core head dims (256)
TOK = B * S
SLAB = 512
CHUNK = 128
GATE_NORM = 16.0
CLAMP_MIN = -50.0


def build_nc(tok=TOK, gate_mode="identity"):
    """Build the per-core SPMD Bass program.

    gate_mode: "identity" -> gate preactivation is x columns (no projection)
               "general"  -> gate = x @ Wg.T + bg computed on device
    """
    assert tok % SLAB == 0 and (tok // B) % SLAB == 0
    nslabs = tok // SLAB
    spb = (tok // B) // SLAB   # slabs per batch
    tpb = tok // B             # tokens per batch
    n_ct = HID // 128          # contraction tiles
    n_tt = SLAB // CHUNK       # token chunks per slab
    n_eo = HID // 512          # output column tiles
    ident_gate = gate_mode == "identity"

    nc = bacc.Bacc(None, target_bir_lowering=False, debug=False)

    xT = nc.dram_tensor("xT", [HID, tok], F16, kind="ExternalInput")
    wqT = nc.dram_tensor("wqT", [HID, DC], F16, kind="ExternalInput")
    wkT = nc.dram_tensor("wkT", [HID, DC], F16, kind="ExternalInput")
    wvT = nc.dram_tensor("wvT", [HID, DC], F16, kind="ExternalInput")
    woT = nc.dram_tensor("woT", [DC, HID], F16, kind="ExternalInput")
    u1 = nc.dram_tensor("u1", [CHUNK, CHUNK], F16, kind="ExternalInput")
    ident = nc.dram_tensor("ident", [CHUNK, CHUNK], F16, kind="ExternalInput")
    if ident_gate:
        xg = nc.dram_tensor("xg", [tok, DC], F16, kind="ExternalInput")
    else:
        wgT = nc.dram_tensor("wgT", [HID, DC], F16, kind="ExternalInput")
        bg = nc.dram_tensor("bg", [1, DC], F16, kind="ExternalInput")
    out = nc.dram_tensor("out", [tok, HID], F16, kind="ExternalOutput")

    # interleave batches: consecutive same-batch slabs have serially
    # dependent scans; alternating keeps two recurrent sweeps in flight.
    slab_order = [bb * spb + si for si in range(spb) for bb in range(B)]

    from contextlib import ExitStack

    with tile.TileContext(nc) as tc, ExitStack() as _st:
        def _pool(name, bufs, space=None):
            kw = {"space": space} if space is not None else {}
            return _st.enter_context(tc.tile_pool(name=name, bufs=bufs, **kw))

        PSUM = bass.MemorySpace.PSUM
        consts = _pool("consts", 1)
        xtp = _pool("xtp", 48)
        gxp = _pool("gxp", 2)
        ggp = _pool("ggp", 2 if ident_gate else 3)
        qkp = _pool("qkp", 3)
        vp = _pool("vp", 8)
        egp = _pool("egp", 4)
        engp = _pool("engp", 3)
        qtp = _pool("qtp", 4)
        ktp = _pool("ktp", 3)
        k2dp = _pool("k2dp", 3)
        k2tp = _pool("k2tp", 16)
        atmp = _pool("atmp", 16)
        attnp = _pool("attnp", 2)
        ostp = _pool("ostp", 2)
        statep = _pool("statep", 2)
        pspp = _pool("ps_pp", 2, PSUM)
        pso = _pool("ps_o", 2, PSUM)
        psgt = _pool("ps_gt", 1, PSUM)
        pssc = _pool("ps_sc", 3, PSUM)
        if True:
            # ---- constants (GpSimd queue: ~25ns dispatch each) ----
            wq_sb = consts.tile([128, n_ct, DC], F16)
            wk_sb = consts.tile([128, n_ct, DC], F16)
            wv_sb = consts.tile([128, n_ct, DC], F16)
            wo_sb = consts.tile([128, HPC, HID], F16)
            # per-ct weight DMAs: dispatch cost scales with descriptor count,
            # so many small dispatches pipeline better than one big one
            for ct in range(n_ct):
                cs = slice(ct * 128, (ct + 1) * 128)
                nc.gpsimd.dma_start(wv_sb[:, ct, :], wvT[cs, :])
            for ct in range(n_ct):
                cs = slice(ct * 128, (ct + 1) * 128)
                nc.gpsimd.dma_start(wq_sb[:, ct, :], wqT[cs, :])
            for ct in range(n_ct):
                cs = slice(ct * 128, (ct + 1) * 128)
                nc.gpsimd.dma_start(wk_sb[:, ct, :], wkT[cs, :])
            u1_sb = consts.tile([CHUNK, CHUNK], F16)
            nc.gpsimd.dma_start(u1_sb, u1[:, :])
            id_sb = consts.tile([CHUNK, CHUNK], F16)
            nc.gpsimd.dma_start(id_sb, ident[:, :])
            nc.gpsimd.dma_start(
                wo_sb, woT[:, :].rearrange("(hp p) e -> p hp e", p=128)
            )
            if not ident_gate:
                wg_sb = consts.tile([128, n_ct, DC], F16)
                nc.gpsimd.dma_start(
                    wg_sb, wgT[:, :].rearrange("(ct p) d -> p ct d", p=128)
                )
                bg_sb = consts.tile([1, DC], F16)
                nc.gpsimd.dma_start(bg_sb, bg[:, :])
                ones1 = consts.tile([1, CHUNK], F16)
                nc.vector.memset(ones1, 1.0)

            # ---- per-(batch, head) recurrent state [dk, dv], fp16 ----
            s_tiles = {}
            for bh in range(B * HPC):
                t = statep.tile([DH, DH], F16, tag=f"S{bh}")
                nc.vector.memset(t.bitcast(F32), 0.0)
                s_tiles[bh] = t

            # ---- x slab tiles (Sync queue) ----
            xts_map = {}

            def load_x(slab):
                ts = []
                t0 = slab * SLAB
                for ct in range(n_ct):
                    t = xtp.tile([128, SLAB], F16, tag="xt")
                    nc.sync.dma_start(t, xT[ct * 128:(ct + 1) * 128, t0:t0 + SLAB])
                    ts.append(t)
                xts_map[slab] = ts

            load_x(slab_order[0])
            load_x(slab_order[1])

            # ---- identity-gate precompute: per-batch x-column tiles,
            #      4 wide softplus ACTs (one table load), per-slab TS ----
            g_tiles = {}
            sp_tiles = {}
            if ident_gate:
                for b in range(B):
                    xg_sb = gxp.tile([128, spb * n_tt, DC], F16, tag="xg")
                    nc.scalar.dma_start(
                        xg_sb,
                        xg[b * tpb:(b + 1) * tpb, :].rearrange(
                            "(c p) d -> p c d", p=128
                        ),
                    )
                    ex_sb = gxp.tile([128, spb * n_tt, DC], F16, tag="ex")
                    ln_sb = gxp.tile([128, spb * n_tt, DC], F16, tag="lns")
                    sp_tiles[b] = (xg_sb, ex_sb, ln_sb)
                    g_tiles[b] = ggp.tile(
                        [128, spb * n_tt, DC], F16, tag="g", name=f"g{b}"
                    )
                half = (spb * n_tt) // 2

                def emit_exp(hi, b):
                    xg_sb, ex_sb, _ = sp_tiles[b]
                    blk = slice(hi * half, (hi + 1) * half)
                    nc.scalar.activation(
                        ex_sb[:, blk, :], xg_sb[:, blk, :], AF.Exp, scale=-1.0
                    )

                def emit_ln(hi, b):
                    # softplus(-z) = ln(1 + exp(-z)); scaling/clamp folded
                    # into the per-slab tensor_scalar below
                    _, ex_sb, ln_sb = sp_tiles[b]
                    blk = slice(hi * half, (hi + 1) * half)
                    nc.scalar.activation(
                        ln_sb[:, blk, :], ex_sb[:, blk, :], AF.Ln, bias=1.0
                    )

                # all gate ACT up front: slab 0's PE-critical copies are on
                # the vector queue (v), so ~15us of gate work on ACT hides
                # behind the v/q/k projections; 3 table loads total
                emit_exp(0, 0)
                emit_exp(0, 1)
                emit_exp(1, 0)
                emit_exp(1, 1)
                emit_ln(0, 0)
                emit_ln(0, 1)
                emit_ln(1, 0)
                emit_ln(1, 1)

            # ---- main slab loop ----
            for idx, slab in enumerate(slab_order):
                b = slab // spb
                lo = slab % spb
                t0 = slab * SLAB
                if idx + 2 < nslabs:
                    load_x(slab_order[idx + 2])
                xts = xts_map.pop(slab)

                if ident_gate:
                    g_b = g_tiles[b]
                    blk = slice(lo * n_tt, (lo + 1) * n_tt)
                    nc.vector.tensor_scalar(
                        g_b[:, blk, :], sp_tiles[b][2][:, blk, :],
                        -1.0 / GATE_NORM, CLAMP_MIN, op0=OP.mult, op1=OP.max,
                    )

                    def gsl(ci, h, _g=g_b, _lo=lo):
                        return _g[:, _lo * n_tt + ci, h * DH:(h + 1) * DH]

                # T-major v projection (out [t, d]); copies on Vector
                v_tiles = []
                for tt in range(n_tt):
                    ps = pspp.tile([128, DC], F32, tag="pp")
                    for ct in range(n_ct):
                        nc.tensor.matmul(
                            ps,
                            xts[ct][:, tt * CHUNK:(tt + 1) * CHUNK],
                            wv_sb[:, ct, :],
                            start=(ct == 0),
                            stop=(ct == n_ct - 1),
                        )
                    v_sb = vp.tile([128, DC], F16, tag="v")
                    nc.vector.tensor_copy(v_sb, ps)
                    v_tiles.append(v_sb)

                # D-major q/k projections (out [d, t])
                qsb, ksb = {}, {}
                for h in range(HPC):
                    for name, wsb, store in (("q", wq_sb, qsb), ("k", wk_sb, ksb)):
                        ps = pspp.tile([128, SLAB], F32, tag="pp")
                        for ct in range(n_ct):
                            nc.tensor.matmul(
                                ps,
                                wsb[:, ct, h * DH:(h + 1) * DH],
                                xts[ct],
                                start=(ct == 0),
                                stop=(ct == n_ct - 1),
                            )
                        sbt = qkp.tile([128, SLAB], F32, tag=name)
                        nc.scalar.copy(sbt, ps)
                        store[h] = sbt

                # general-gate: z = x @ Wg.T + bg, T-major, then softplus
                if not ident_gate:
                    g_t = ggp.tile([128, n_tt, DC], F16, tag="g")

                    def gsl(ci, h, _g=g_t):
                        return _g[:, ci, h * DH:(h + 1) * DH]

                    for tt in range(n_tt):
                        zps = pspp.tile([128, DC], F32, tag="pp")
                        for ct in range(n_ct):
                            nc.tensor.matmul(
                                zps,
                                xts[ct][:, tt * CHUNK:(tt + 1) * CHUNK],
                                wg_sb[:, ct, :],
                                start=(ct == 0),
                                stop=False,
                            )
                        nc.tensor.matmul(zps, ones1, bg_sb, start=False, stop=True)
                        spe = gxp.tile([128, DC], F16, tag="spe")
                        nc.scalar.activation(spe, zps, AF.Exp, scale=-1.0)
                        spt = gxp.tile([128, DC], F16, tag="spt")
                        nc.scalar.activation(spt, spe, AF.Ln, bias=1.0)
                        nc.vector.tensor_scalar(
                            g_t[:, tt, :], spt,
                            -1.0 / GATE_NORM, CLAMP_MIN, op0=OP.mult, op1=OP.max,
                        )

                # G cumsum matmuls + exps: expG/expNG assembled per head
                # into [128, SLAB] tiles so qt/kt are single wide muls
                egs, engs = {}, {}
                for h in range(HPC):
                    egA = egp.tile([128, SLAB], F32, tag="eg")
                    engA = engp.tile([128, SLAB], F32, tag="eng")
                    for ci in range(n_tt):
                        gps = psgt.tile([DH, CHUNK], F32, tag="gt")
                        nc.tensor.matmul(gps, gsl(ci, h), u1_sb, start=True, stop=True)
                        cs = slice(ci * CHUNK, (ci + 1) * CHUNK)
                        nc.scalar.activation(egA[:, cs], gps, AF.Exp)
                        nc.scalar.activation(engA[:, cs], gps, AF.Exp, scale=-1.0)
                    egs[h], engs[h] = egA, engA

                # decayed q/k + per-chunk K2 transpose and AT
                qts, k2ts, atms = {}, {}, {}
                for h in range(HPC):
                    qtA = qtp.tile([128, SLAB], F16, tag="qt")
                    nc.vector.tensor_mul(qtA, qsb[h], egs[h])
                    ktA = ktp.tile([128, SLAB], F16, tag="kt")
                    nc.vector.tensor_mul(ktA, ksb[h], engs[h])
                    qts[h] = qtA
                    for ci in range(n_tt):
                        cs = slice(ci * CHUNK, (ci + 1) * CHUNK)
                        last = slice(ci * CHUNK + CHUNK - 1, ci * CHUNK + CHUNK)
                        k2d = k2dp.tile([DH, CHUNK], F16, tag="k2d")
                        nc.vector.tensor_scalar_mul(k2d, ktA[:, cs], egs[h][:, last])
                        tps = psgt.tile([CHUNK, DH], F16, tag="gt")
                        nc.tensor.transpose(tps, k2d, id_sb)
                        k2t = k2tp.tile([CHUNK, DH], F16, tag="k2t")
                        nc.vector.tensor_copy(k2t, tps)
                        k2ts[(h, ci)] = k2t
                        aps = pssc.tile([CHUNK, CHUNK], F32, tag="sc")
                        nc.tensor.matmul(aps, ktA[:, cs], qtA[:, cs], start=True, stop=True)
                        atm = atmp.tile([CHUNK, CHUNK], F16, tag="atm")
                        nc.vector.tensor_mul(atm, aps, u1_sb)
                        atms[(h, ci)] = atm

                # recurrent sweep (chunk-serial per head) + o_proj per chunk
                attn_tiles = {}
                for h in range(HPC):
                    attn_tiles[h] = attnp.tile(
                        [DH, SLAB], F16, tag=f"at{h}", name=f"attn{h}"
                    )
                for ci in range(n_tt):
                    cs = slice(ci * CHUNK, (ci + 1) * CHUNK)
                    last = slice(ci * CHUNK + CHUNK - 1, ci * CHUNK + CHUNK)
                    for h in range(HPC):
                        bh = b * HPC + h
                        s_old = s_tiles[bh]
                        v_T = v_tiles[ci][:, h * DH:(h + 1) * DH]
                        ot = pssc.tile([DH, CHUNK], F32, tag="sc")
                        nc.tensor.matmul(ot, s_old, qts[h][:, cs], start=True, stop=False)
                        nc.tensor.matmul(ot, v_T, atms[(h, ci)], start=False, stop=True)
                        nc.scalar.copy(attn_tiles[h][:, cs], ot)
                        kvp = pssc.tile([DH, DH], F32, tag="sc")
                        nc.tensor.matmul(kvp, k2ts[(h, ci)], v_T, start=True, stop=True)
                        s_new = statep.tile([DH, DH], F16, tag=f"S{bh}")
                        nc.vector.scalar_tensor_tensor(
                            s_new, s_old, egs[h][:, last], kvp,
                            op0=OP.mult, op1=OP.add,
                        )
                        s_tiles[bh] = s_new

                    # row-parallel o_proj for this chunk's tokens
                    ostage = ostp.tile([CHUNK, HID], F16, tag="os")
                    for eo in range(n_eo):
                        ops = pso.tile([CHUNK, 512], F32, tag="o")
                        for h in range(HPC):
                            nc.tensor.matmul(
                                ops,
                                attn_tiles[h][:, cs],
                                wo_sb[:, h, eo * 512:(eo + 1) * 512],
                                start=(h == 0),
                                stop=(h == HPC - 1),
                            )
                        dst = ostage[:, eo * 512:(eo + 1) * 512]
                        if eo % 2:
                            nc.scalar.copy(dst, ops)
                        else:
                            nc.vector.tensor_copy(dst, ops)
                    nc.gpsimd.dma_start(
                        out[t0 + ci * CHUNK:t0 + (ci + 1) * CHUNK, :], ostage
                    )
    nc.compile()
    return nc


_NC_CACHE = {}
LAST_RESULTS = []


def _get_nc(tok, gate_mode):
    key = (tok, gate_mode)
    if key not in _NC_CACHE:
        _NC_CACHE[key] = build_nc(tok, gate_mode)
    return _NC_CACHE[key]


def _make_in_maps(xT, x, Wq, Wk, Wv, Wo, gate_mode, Wg=None, bgv=None, tok=TOK):
    scale = DH ** -0.5
    u1m = np.triu(np.ones((CHUNK, CHUNK), np.float16))
    idm = np.eye(CHUNK, dtype=np.float16)
    xTp = np.ascontiguousarray(xT.astype(np.float16))
    in_maps = []
    for c in range(NCORES):
        rs = slice(c * DC, (c + 1) * DC)
        m = dict(
            xT=xTp,
            wqT=np.ascontiguousarray((Wq[rs] * scale).T.astype(np.float16)),
            wkT=np.ascontiguousarray(Wk[rs].T.astype(np.float16)),
            wvT=np.ascontiguousarray(Wv[rs].T.astype(np.float16)),
            woT=np.ascontiguousarray(Wo[:, rs].T.astype(np.float16)),
            u1=u1m,
            ident=idm,
        )
        if gate_mode == "identity":
            m["xg"] = np.ascontiguousarray(x[:, rs].astype(np.float16))
        else:
            m["wgT"] = np.ascontiguousarray(Wg[rs].T.astype(np.float16))
            m["bg"] = np.ascontiguousarray(bgv[rs].astype(np.float16)).reshape(1, DC)
        in_maps.append(m)
    return in_maps


def _run(nc, in_maps):
    trace = bool(int(os.environ.get("GLA_TRACE", "0")))
    res = run_bass_kernel_spmd(
        nc, in_maps, list(range(NCORES)), trace=trace,
    )
    LAST_RESULTS.append(res)
    total = res.results[0]["out"].astype(np.float32)
    for i in range(1, NCORES):
        total += res.results[i]["out"].astype(np.float32)
    return total


def kernel(hidden_states, Wq, Wk, Wv, Wo, Wg1, bg1, Wg2, bg2, alpha_list):
    LAST_RESULTS.clear()
    x = np.ascontiguousarray(np.asarray(hidden_states, np.float32).reshape(TOK, HID))
    xT = np.ascontiguousarray(x.T)
    Wq = np.asarray(Wq, np.float32)
    Wk = np.asarray(Wk, np.float32)
    Wv = np.asarray(Wv, np.float32)
    Wo = np.asarray(Wo, np.float32)
    Wg1 = np.asarray(Wg1, np.float32)
    Wg2 = np.asarray(Wg2, np.float32)
    bg1 = np.asarray(bg1, np.float32)
    bg2 = np.asarray(bg2, np.float32)
    al = np.asarray(alpha_list, np.float64)
    a = np.exp(al - al.max())
    a = (a / a.sum()).astype(np.float32)

    gates_equal = np.array_equal(Wg1, Wg2) and np.array_equal(bg1, bg2)
    ident_gate = (
        gates_equal
        and not bg1.any()
        and np.array_equal(Wg1, np.eye(HID, dtype=np.float32))
    )

    if ident_gate:
        nc = _get_nc(TOK, "identity")
        out = _run(nc, _make_in_maps(xT, x, Wq, Wk, Wv, Wo, "identity"))
    elif gates_equal:
        nc = _get_nc(TOK, "general")
        out = _run(nc, _make_in_maps(xT, x, Wq, Wk, Wv, Wo, "general", Wg1, bg1))
    else:
        nc = _get_nc(TOK, "general")
        o1 = _run(nc, _make_in_maps(xT, x, Wq, Wk, Wv, Wo, "general", Wg1, bg1))
        o2 = _run(nc, _make_in_maps(xT, x, Wq, Wk, Wv, Wo, "general", Wg2, bg2))
        out = a[0] * o1 + a[1] * o2

    return out.reshape(B, S, HID)
